# revision 1
# baseline (speedup 1.0000x reference)
"""Bipartite NAND/NOR graph layer on 8 Trainium2 NeuronCores.

Problem: out[i] = ~(x[a_i] & x[b_i]) if not nor_mask[i] else ~(x[a_i] | x[b_i])
with x: [32768, 2048] int32, (a, b): [32768, 2] indices, nor_mask: [32768] bool.

Strategy (2-way output split x 4-way word split, zero cross-core traffic):
- The 2048-word axis is cut into 4 slices of 512 words (2 KiB); the output
  rows are cut into 2 halves.  Core c owns word-slice c % 4 of output-half
  c // 4.  Per core that means ~16.4K outputs -> ~33K gather descriptors of
  2 KiB each, which halves the gpsimd descriptor-generation time (the
  bottleneck of the 8-way word-split layout) and doubles DMA descriptor size.
- Host passes the COMPLEMENTED table cx = ~x.  By De Morgan:
      NAND: ~(a & b) = cx_a | cx_b      NOR: ~(a | b) = cx_a & cx_b
  so each output row is one OR/AND of two gathered complemented rows.
- Output rows are grouped by mask value and sorted by min-index; each group
  is split into two equal halves (first half -> cores 0-3, second -> 4-7) so
  BOTH halves see the identical chunk structure (same ops, same num_idxs per
  chunk) and a single SPMD program serves all 8 cores; only the index
  tables and word-slice data differ per core.
- Chunks are CHUNK rows (full) plus one exact-size tail chunk per group, so
  padding waste is <16 rows per group.  Gathers use the gpsimd dma_gather
  ucode (int16 indices wrapped in 16 partitions): row j of a chunk lands at
  SBUF [j % 128, j // 128, :].  The device stores output chunks
  partition-major ([128, ...] contiguous per partition -> 16 KiB HBM write
  bursts); the host untransposes and scatters rows back.
"""
import sys
sys.path.insert(0, "/opt/trn_rl_repo")

import numpy as np
from contextlib import ExitStack

import concourse.bass as bass
import concourse.tile as tile
from concourse import bacc, mybir
from concourse.bass_utils import run_bass_kernel_spmd

N_ROWS = 32768          # input rows == output rows
W_FULL = 2048           # int32 words per row
N_CORES = 8
WORD_SPLIT = 4
OUT_SPLIT = 2
WS = W_FULL // WORD_SPLIT   # 512 words per core slice (2 KiB)
P = 128
CHUNK = 1024            # rows per full dma_gather call (ucode limit: 2048 crashes)
# The SDMA engines round-robin DMA rings at packet granularity, which
# starves small-packet rings next to big-packet ones (HWDGE writes got
# ~10% next to the 128 KiB gather packets, stalling the pipeline on
# r-buffer reuse).  Three SWDGE rings instead: gathers on rings 1/2,
# writes alone on ring 0 with single_packet=True so their packets
# round-robin at the same weight as the gathers'.  Writes are emitted 2
# chunks behind the gathers so their vector dependency is long resolved
# and the in-order Pool engine never stalls on them.
CHUNK_QUEUES = ((1, 2), (1, 2))   # indexed by chunk parity
NUM_SWDGE_QUEUES = 3
WRITE_LAG = 2


def _wrap_idxs(idx_chunk):
    """[n] int -> [128, ceil(n/16)] int16 wrapped in 16 partitions, replicated
    across the 8 gpsimd core windows.  Pads to a multiple of 16 with 0 (the
    pad never enters the gather: num_idxs < pad position)."""
    n = len(idx_chunk)
    cols = -(-n // 16)
    padded = np.zeros(cols * 16, dtype=np.int16)
    padded[:n] = idx_chunk.astype(np.int16)
    blk = padded.reshape(cols, 16).T  # [16, cols]
    return np.tile(blk, (8, 1))


def _prepare(output_node_input_indices, nor_mask):
    """Group rows by mask, sort by min index, split into 2 equal halves.

    Returns (ias, ibs, chunks, row_orders) where
      chunks     = [(op, num_idxs), ...] shared by both halves,
      ias/ibs[o] = wrapped int16 index planes for half o,
      row_orders[o][k] = original output row stored at device row k
                         (-1 for rows the host must ignore)."""
    idx = np.asarray(output_node_input_indices)
    mask = np.asarray(nor_mask).astype(bool)
    # AND/OR are commutative: put the smaller index in operand a, then order
    # rows by it.  The a-gather then reads HBM nearly sequentially.
    lo = np.minimum(idx[:, 0], idx[:, 1]).astype(np.int64)
    hi = np.maximum(idx[:, 0], idx[:, 1]).astype(np.int64)

    chunks = []                     # (op, num_idxs) — identical across halves
    half_rows = [[], []]            # per-half list of per-chunk row arrays
    groups = [(np.flatnonzero(sel), op)
              for sel, op in ((~mask, 'or'), (mask, 'and'))]
    groups = [(r, op) for r, op in groups if len(r)]
    for gi, (rows, op) in enumerate(groups):
        rows = rows[np.argsort(lo[rows], kind="stable")]
        H = -(-len(rows) // 2)      # per-half count (half 1 padded w/ dups)
        halves = [rows[:H], rows[H:]]
        if len(halves[1]) < H:
            halves[1] = np.concatenate(
                [halves[1], np.repeat(rows[-1], H - len(halves[1]))])
        n_full, tail = divmod(H, CHUNK)
        sizes = [CHUNK] * n_full + ([tail] if tail else [])
        if gi == len(groups) - 1:
            # Split the final chunk into descending pieces: less work is
            # outstanding when the pipeline drains, shrinking the serial
            # tail (last gather -> vector -> write) of the kernel.
            t = sizes.pop()
            for piece in (512, 256):
                if t > piece + 128:
                    sizes.append(piece)
                    t -= piece
            sizes.append(t)
        chunks.extend((op, s) for s in sizes)
        for o in (0, 1):
            off = 0
            for s in sizes:
                half_rows[o].append(halves[o][off:off + s])
                off += s

    ias, ibs, row_orders = [], [], []
    for o in (0, 1):
        ia_planes, ib_planes, ro = [], [], []
        for rows_c in half_rows[o]:
            ia_planes.append(_wrap_idxs(lo[rows_c]))
            ib_planes.append(_wrap_idxs(hi[rows_c]))
            # device rows per chunk = ceil(n/128)*128; host ignores the pad
            nb = -(-len(rows_c) // P) * P
            ro_c = np.full(nb, -1, dtype=np.int64)
            ro_c[:len(rows_c)] = rows_c
            ro.append(ro_c)
        ias.append(np.concatenate(ia_planes, axis=1))
        ibs.append(np.concatenate(ib_planes, axis=1))
        row_orders.append(np.concatenate(ro))
    return ias, ibs, chunks, row_orders


def _build(chunks):
    """chunks = [(op, num_idxs), ...]; one SPMD program for all 8 cores."""
    idx_cols = sum(-(-n // 16) for _, n in chunks)
    out_free = sum(-(-n // P) * WS for _, n in chunks)
    nc = bacc.Bacc("TRN2", target_bir_lowering=False, debug=False,
                   num_devices=N_CORES, num_swdge_queues=NUM_SWDGE_QUEUES)
    x = nc.dram_tensor("x", [N_ROWS, WS], mybir.dt.int32,
                       kind="ExternalInput").ap()
    ia = nc.dram_tensor("ia", [P, idx_cols], mybir.dt.int16,
                        kind="ExternalInput").ap()
    ib = nc.dram_tensor("ib", [P, idx_cols], mybir.dt.int16,
                        kind="ExternalInput").ap()
    out = nc.dram_tensor("out", [P, out_free], mybir.dt.int32,
                         kind="ExternalOutput").ap()
    with ExitStack() as ctx:
        tc = ctx.enter_context(tile.TileContext(nc))
        idxp = ctx.enter_context(tc.tile_pool(name="idx", bufs=1))
        datap = ctx.enter_context(tc.tile_pool(name="data", bufs=3))
        ta_i = idxp.tile([P, idx_cols], mybir.dt.int16)
        tb_i = idxp.tile([P, idx_cols], mybir.dt.int16)
        # Split the index load so the first chunk's gathers only wait for a
        # small leading slice, not the whole table.
        c0 = min(-(-chunks[0][1] // 16), idx_cols)
        nc.sync.dma_start(ta_i[:, :c0], ia[:, :c0])
        nc.scalar.dma_start(tb_i[:, :c0], ib[:, :c0])
        if c0 < idx_cols:
            nc.sync.dma_start(ta_i[:, c0:], ia[:, c0:])
            nc.scalar.dma_start(tb_i[:, c0:], ib[:, c0:])
        icol = 0
        ocol = 0
        pending = []
        for ci, (op, n) in enumerate(chunks):
            qa, qb = CHUNK_QUEUES[ci % 2]
            cols = -(-n // 16)
            b = -(-n // P)
            isl = slice(icol, icol + cols)
            icol += cols
            ta = datap.tile([P, CHUNK // P, WS], mybir.dt.int32, tag="ta")
            nc.gpsimd.dma_gather(
                out_ap=ta[:, :b, :], in_ap=x, idxs_ap=ta_i[:, isl],
                num_idxs=n, num_idxs_reg=n,
                elem_size=WS, queue_num=qa)
            # Issue the lagged write BETWEEN the two gathers: its descriptor
            # generation gives ring 2 extra drain time before b is pushed,
            # locking in the good phase (a blocks on ring 1, b never does).
            if len(pending) >= WRITE_LAG:
                o_ap, r_ap = pending.pop(0)
                nc.gpsimd.dma_start(o_ap, r_ap, single_packet=True)
            tb = datap.tile([P, CHUNK // P, WS], mybir.dt.int32, tag="tb")
            nc.gpsimd.dma_gather(
                out_ap=tb[:, :b, :], in_ap=x, idxs_ap=tb_i[:, isl],
                num_idxs=n, num_idxs_reg=n,
                elem_size=WS, queue_num=qb)
            r = datap.tile([P, CHUNK // P, WS], mybir.dt.int32, tag="r", bufs=4)
            alu = (mybir.AluOpType.bitwise_or if op == 'or'
                   else mybir.AluOpType.bitwise_and)
            nc.vector.tensor_tensor(out=r[:, :b, :], in0=ta[:, :b, :],
                                    in1=tb[:, :b, :], op=alu)
            pending.append((
                out[:, ocol:ocol + b * WS].rearrange(
                    'p (b w) -> p b w', b=b, w=WS),
                r[:, :b, :]))
            ocol += b * WS
        for o_ap, r_ap in pending:
            nc.gpsimd.dma_start(o_ap, r_ap, single_packet=True)
    nc.finalize()
    return nc


def _in_maps(input_bitarrays, ias, ibs):
    cx = ~np.asarray(input_bitarrays)  # complemented table (De Morgan)
    slices = [np.ascontiguousarray(cx[:, w * WS:(w + 1) * WS])
              for w in range(WORD_SPLIT)]
    return [{"x": slices[c % WORD_SPLIT], "ia": ias[c // WORD_SPLIT],
             "ib": ibs[c // WORD_SPLIT]} for c in range(N_CORES)]


def kernel(input_bitarrays, output_node_input_indices, nor_mask):
    x = np.asarray(input_bitarrays)
    assert x.shape == (N_ROWS, W_FULL) and x.dtype == np.int32
    ias, ibs, chunks, row_orders = _prepare(
        output_node_input_indices, nor_mask)
    nc = _build(chunks)
    res = run_bass_kernel_spmd(nc, _in_maps(x, ias, ibs),
                               core_ids=list(range(N_CORES)))

    blocks = [-(-n // P) for _, n in chunks]
    result = np.empty((N_ROWS, W_FULL), dtype=np.int32)
    for c in range(N_CORES):
        o, w = c // WORD_SPLIT, c % WORD_SPLIT
        ro = row_orders[o]
        arr = res.results[c]["out"]  # [128, sum(b)*WS]
        # device row (chunk c, block b, partition p) lives at
        # arr[p, (boff+b)*WS : ...]; flatten back to [rows, WS]
        rows = np.empty((len(ro), WS), dtype=np.int32)
        boff = 0
        roff = 0
        for b in blocks:
            blk = arr[:, boff * WS:(boff + b) * WS].reshape(P, b, WS)
            rows[roff:roff + b * P] = blk.transpose(1, 0, 2).reshape(-1, WS)
            boff += b
            roff += b * P
        valid = ro >= 0
        result[ro[valid], w * WS:(w + 1) * WS] = rows[valid]
    return result

